# revision 7
# baseline (speedup 1.0000x reference)
"""MoE clustered attention kernel for Trainium2 (8 NeuronCores).

Problem: B=2, LQ=LK=2048, D=1024, H=16 heads (DH=64), M=8 clusters.
Each query/key token is routed (argmax of X @ Wr) to one of 8 clusters;
attention is only computed within a cluster (block-sparse attention).

Strategy (v3)
-------------
Host side:
  * compute router assignments with numpy fp32,
  * gather tokens by cluster into padded contiguous ranges (queries to
    >=256 and even, keys to multiples of 128) so one SPMD program
    serves both batches,
  * pre-transpose X to [D, L]; pre-pack weights into the on-chip
    [128, 2048] layout so weight DMAs are fully contiguous,
  * append 9 "mask rows" to the per-head qT/kT tensors: the scores
    matmul contracts over 64+9=73 rows and the extra rows add exactly
    0 to same-cluster pairs and exactly -16384 to cross-cluster or
    padded pairs (exp(x-16384) == 0).

Device side (per core; core = batch * 4 + head_group, 4 heads each):
  * per-head qT/kT projections in transposed layout [73, L] with N=512
    moving matmuls; v in natural layout with an appended ones column
    that makes the ctx matmul emit the softmax denominator as row 64,
  * attention iterates cluster-outer / head-inner; per (cluster, head):
    scores into 2-bank PSUM supertiles, exp on ScalarE, ctx matmul,
    then Ln of the denominator row (ScalarE, doubles as the staging
    copy to partition 0) and an unnormalized ctx evacuation (VectorE),
  * per cluster: one batched exp(-x) over the 4 heads' ln-rows gives
    the reciprocals; GpSimd broadcasts each head's row and multiplies
    the ctx block in place (keeping the normalize entirely off the
    Tensor/Scalar/Vector critical path),
  * output projection is interleaved: as soon as a cluster's token
    range is normalized for all 4 heads, the covered 128-token chunks
    are projected (N=512 matmuls) and DMA'd out in bf16.
  * ACT tables are patched so Exp and Ln both resolve to the
    natural_log_exp_and_others set: one table load, no thrashing.
Host sums the 4 head-group partials per batch in fp32 and un-permutes.
"""

import numpy as np
import ml_dtypes

import concourse.bacc as bacc
import concourse.tile as tile
import concourse.mybir as mybir
import concourse.hw_specs as hw_specs
from concourse.bass_utils import run_bass_kernel_spmd

F32 = mybir.dt.float32
BF16 = mybir.dt.bfloat16
EXP = mybir.ActivationFunctionType.Exp
LN = mybir.ActivationFunctionType.Ln
MULT = mybir.AluOpType.mult

H = 16            # total heads
HPC = 4           # heads per core
N_CORES = 8
SQRT_BIG = 128.0  # sqrt(16384); mask contributions are exact powers of two

MMDT = BF16
NPDT = ml_dtypes.bfloat16

# Route Exp and Ln to the one ACT table set that contains both, so the
# table-load insertion pass emits a single load instead of thrashing
# between exp_and_others and natural_log on every call.
_orig_get_activation_tables = hw_specs.get_activation_tables


def _patched_get_activation_tables(arch):
    out = {}
    for name, fns in _orig_get_activation_tables(arch).items():
        fns = set(fns)
        if name != "natural_log_exp_and_others":
            fns.discard(EXP)
            fns.discard(LN)
        out[name] = fns
    return out


bacc.get_activation_tables = _patched_get_activation_tables


def _ceil_to(x, m):
    return (x + m - 1) // m * m


def _plan(aq, ak, M):
    """Common (cross-batch) padded cluster geometry."""
    B = aq.shape[0]
    nq = np.array([[int((aq[b] == c).sum()) for c in range(M)] for b in range(B)])
    nk = np.array([[int((ak[b] == c).sum()) for c in range(M)] for b in range(B)])
    NQP = [max(256, _ceil_to(int(nq[:, c].max()), 2)) for c in range(M)]
    NKP = [_ceil_to(max(128, int(nk[:, c].max())), 128) for c in range(M)]
    qoff = np.concatenate([[0], np.cumsum(NQP)])
    koff = np.concatenate([[0], np.cumsum(NKP)])
    LQG = _ceil_to(int(qoff[-1]), 256)
    NKG = _ceil_to(int(koff[-1]), 256)
    return NQP, NKP, qoff[:-1].tolist(), koff[:-1].tolist(), LQG, NKG


def _build_program(NQP, NKP, qoffs, koffs, LQG, NKG, D):
    nc = bacc.Bacc("TRN2", target_bir_lowering=False, debug=False)
    XQT = nc.dram_tensor("XQT", [D, LQG], MMDT, kind="ExternalInput").ap()
    XKT = nc.dram_tensor("XKT", [D, NKG], MMDT, kind="ExternalInput").ap()
    XVT = nc.dram_tensor("XVT", [D, NKG], MMDT, kind="ExternalInput").ap()
    WQ = nc.dram_tensor("WQ", [128, 2048], MMDT, kind="ExternalInput").ap()
    WK = nc.dram_tensor("WK", [128, 2048], MMDT, kind="ExternalInput").ap()
    WV = nc.dram_tensor("WV", [128, 2048], MMDT, kind="ExternalInput").ap()
    WO = nc.dram_tensor("WO", [128, 2048], MMDT, kind="ExternalInput").ap()
    MQ = nc.dram_tensor("MQ", [9, LQG], MMDT, kind="ExternalInput").ap()
    MK = nc.dram_tensor("MK", [9, NKG], MMDT, kind="ExternalInput").ap()
    OUT = nc.dram_tensor("OUT", [LQG, D], BF16, kind="ExternalOutput").ap()

    ND = D // 128          # contraction chunks (8)
    NVC = NKG // 128       # value token chunks
    M = len(NQP)

    with tile.TileContext(nc) as tc:
        with (
            tc.tile_pool(name="weights", bufs=1) as wpool,
            tc.tile_pool(name="proj_out", bufs=1) as projpool,
            tc.tile_pool(name="psA", bufs=2, space="PSUM") as psA,
            tc.tile_pool(name="psB", bufs=2, space="PSUM") as psB,
            tc.tile_pool(name="psC", bufs=2, space="PSUM") as psC,
        ):
            wq = wpool.tile([128, 2048], MMDT, tag="wq")
            wk = wpool.tile([128, 2048], MMDT, tag="wk")
            wv = wpool.tile([128, 2048], MMDT, tag="wv")
            wo = wpool.tile([128, 2048], MMDT, tag="wo")

            qT = [projpool.tile([73, LQG], MMDT, tag=f"qT{h}", name=f"qT{h}")
                  for h in range(HPC)]
            kT = [projpool.tile([73, NKG], MMDT, tag=f"kT{h}", name=f"kT{h}")
                  for h in range(HPC)]
            vA = projpool.tile([128, NVC * 260], MMDT, tag="vA")

            # first weight first: Q proj's inputs arrive earliest
            nc.sync.dma_start(wq[:], WQ)

            nc.vector.memset(vA[:], 1.0)

            # ---- projections ----
            with tc.tile_pool(name="xin", bufs=3) as xpool:
                def proj_T(xdram, L, wtile, dest):
                    """dest[h][0:64, L] = (W_h.T @ X^T), streamed over L."""
                    for off in range(0, L, 512):
                        w = min(512, L - off)
                        xt = xpool.tile([128, ND, 512], MMDT, tag="xt")
                        nc.sync.dma_start(
                            xt[:, :, :w],
                            xdram.rearrange("(n p) m -> p n m", p=128)[:, :, off:off + w])
                        for pair in range(2):
                            ps = psA.tile([128, 512], F32, tag="psproj")
                            for d in range(ND):
                                nc.tensor.matmul(
                                    ps[:, :w],
                                    wtile[:, d * 256 + pair * 128: d * 256 + (pair + 1) * 128],
                                    xt[:, d, :w],
                                    start=(d == 0), stop=(d == ND - 1))
                            if pair == 0:
                                nc.vector.tensor_copy(dest[0][0:64, off:off + w], ps[0:64, :w])
                                nc.scalar.copy(dest[1][0:64, off:off + w], ps[64:128, :w])
                            else:
                                nc.scalar.copy(dest[2][0:64, off:off + w], ps[0:64, :w])
                                nc.vector.tensor_copy(dest[3][0:64, off:off + w], ps[64:128, :w])

                proj_T(XQT, LQG, wq, qT)
                # mask rows (needed by scores, not by projections)
                for h in range(HPC):
                    nc.sync.dma_start(qT[h][64:73, :], MQ)
                    nc.sync.dma_start(kT[h][64:73, :], MK)
                nc.sync.dma_start(wk[:], WK)
                proj_T(XKT, NKG, wk, kT)

                nc.sync.dma_start(wv[:], WV)
                nc.sync.dma_start(wo[:], WO)
                for off in range(0, NKG, 512):
                    w = min(512, NKG - off)
                    xt = xpool.tile([128, ND, 512], MMDT, tag="xt")
                    nc.sync.dma_start(
                        xt[:, :, :w],
                        XVT.rearrange("(n p) m -> p n m", p=128)[:, :, off:off + w])
                    for sub in range(w // 128):
                        tc128 = off // 128 + sub
                        ps = psA.tile([128, 512], F32, tag="psproj")
                        for d in range(ND):
                            nc.tensor.matmul(ps[:, 0:256],
                                             xt[:, d, sub * 128:(sub + 1) * 128],
                                             wv[:, d * 256:(d + 1) * 256],
                                             start=(d == 0), stop=(d == ND - 1))
                        nc.vector.tensor_copy(
                            vA[:].rearrange("p (c h e) -> p c h e", c=NVC, h=HPC)[:, tc128, :, 0:64],
                            ps[:, 0:256].rearrange("p (h e) -> p h e", h=HPC))

            # ---- clustered attention + interleaved output projection ----
            # ctx lives in small per-cluster tiles so cluster c's output
            # projection never write-after-read blocks cluster c+1's ctx.
            with tc.tile_pool(name="epool", bufs=3) as epool, \
                 tc.tile_pool(name="ccpool", bufs=3) as ccpool, \
                 tc.tile_pool(name="btpool", bufs=2) as btpool, \
                 tc.tile_pool(name="bbpool", bufs=4) as bbpool, \
                 tc.tile_pool(name="outsb", bufs=4) as opool:
                for c in range(M):
                    qo, nqp = qoffs[c], NQP[c]
                    nkc = NKP[c] // 128
                    lnrow = btpool.tile([1, HPC * 512], F32, tag="lnrow")
                    rcp = btpool.tile([1, HPC * 512], F32, tag="rcp")
                    ctxc = [ccpool.tile([128, 512], MMDT, tag=f"cc{p}", name=f"cc{p}")
                            for p in range(2)]
                    for h in range(HPC):
                        pair, rb = h // 2, (h % 2) * 64
                        es, eslice = [], []
                        for ki in range(0, nkc, 2):
                            nk2 = min(2, nkc - ki)
                            ps_s = psB.tile([128, 1024], F32, tag="ps_s")
                            e = epool.tile([128, 1024], MMDT, tag="e")
                            for kj in range(nk2):
                                ko = koffs[c] + (ki + kj) * 128
                                nc.tensor.matmul(
                                    ps_s[:, kj * 512: kj * 512 + nqp],
                                    kT[h][0:73, ko:ko + 128],
                                    qT[h][0:73, qo:qo + nqp],
                                    start=True, stop=True)
                                es.append(e)
                                eslice.append(slice(kj * 512, kj * 512 + nqp))
                            pv = ps_s[:].rearrange("p (b n) -> p b n", b=2)[:, 0:nk2, 0:nqp]
                            ev = e[:].rearrange("p (b n) -> p b n", b=2)[:, 0:nk2, 0:nqp]
                            nc.scalar.activation(ev, pv, EXP)
                        ps_c = psC.tile([128, 512], F32, tag="ps_c")
                        for ki in range(nkc):
                            kc128 = koffs[c] // 128 + ki
                            nc.tensor.matmul(ps_c[:65, :nqp],
                                             vA[:, kc128 * 260 + h * 65: kc128 * 260 + (h + 1) * 65],
                                             es[ki][:, eslice[ki]],
                                             start=(ki == 0), stop=(ki == nkc - 1))
                        # ln(denominator) doubles as the staging copy to
                        # partition 0; ctx rows leave PSUM unnormalized
                        nc.scalar.activation(lnrow[:, h * 512: h * 512 + nqp],
                                             ps_c[64:65, :nqp], LN)
                        nc.vector.tensor_copy(ctxc[pair][rb:rb + 64, :nqp],
                                              ps_c[0:64, :nqp])
                    # one batched exp(-x) -> reciprocals for all 4 heads
                    nc.scalar.activation(
                        rcp[:].rearrange("p (h n) -> p h n", h=HPC)[:, :, :nqp],
                        lnrow[:].rearrange("p (h n) -> p h n", h=HPC)[:, :, :nqp],
                        EXP, scale=-1.0)
                    for h in range(HPC):
                        pair, rb = h // 2, (h % 2) * 64
                        bt = bbpool.tile([128, 512], F32, tag="bt")
                        nc.gpsimd.partition_broadcast(
                            bt[:, :nqp], rcp[:, h * 512: h * 512 + nqp])
                        nc.gpsimd.tensor_tensor(
                            ctxc[pair][rb:rb + 64, :nqp],
                            ctxc[pair][rb:rb + 64, :nqp],
                            bt[rb:rb + 64, :nqp], MULT)
                    # output projection of this cluster's token range
                    for j in range(0, nqp, 128):
                        w = min(128, nqp - j)
                        ob = opool.tile([128, 1024], BF16, tag="ob")
                        for n2 in range(2):
                            ps_o = psA.tile([128, 512], F32, tag="psproj")
                            for pair in range(2):
                                nc.tensor.matmul(
                                    ps_o[:w, :],
                                    ctxc[pair][:, j:j + w],
                                    wo[:, pair * 1024 + n2 * 512:
                                       pair * 1024 + (n2 + 1) * 512],
                                    start=(pair == 0), stop=(pair == 1))
                            if n2:
                                nc.scalar.copy(ob[:w, n2 * 512:(n2 + 1) * 512],
                                               ps_o[:w, :])
                            else:
                                nc.vector.tensor_copy(ob[:w, n2 * 512:(n2 + 1) * 512],
                                                      ps_o[:w, :])
                        nc.sync.dma_start(OUT[qo + j: qo + j + w, :], ob[:w, :])

    nc.compile()
    return nc


_CACHE = {}


def run(inputs, trace=False):
    queries = np.asarray(inputs["queries"], np.float32)
    keys = np.asarray(inputs["keys"], np.float32)
    values = np.asarray(inputs["values"], np.float32)
    Wq = np.asarray(inputs["Wq"], np.float32)
    Wk = np.asarray(inputs["Wk"], np.float32)
    Wv = np.asarray(inputs["Wv"], np.float32)
    Wo = np.asarray(inputs["Wo"], np.float32)
    Wr = np.asarray(inputs["Wr"], np.float32)

    B, LQ, D = queries.shape
    M = Wr.shape[1]
    DH = D // H
    scale = np.float32(1.0 / np.sqrt(DH))

    aq = np.argmax(queries @ Wr, axis=-1)   # [B, LQ]
    ak = np.argmax(keys @ Wr, axis=-1)      # [B, LK]

    NQP, NKP, qoffs, koffs, LQG, NKG = _plan(aq, ak, M)

    key = (tuple(NQP), tuple(NKP), LQG, NKG, D, str(MMDT))
    if key not in _CACHE:
        _CACHE[key] = _build_program(NQP, NKP, qoffs, koffs, LQG, NKG, D)
    nc = _CACHE[key]

    # ---- gather + pad, build per-batch inputs ----
    perm_q = []   # original token ids, per batch, in gathered order
    slot_q = []   # gathered positions of those tokens
    XQTs, XKTs, XVTs, MQs, MKs = [], [], [], [], []
    for b in range(B):
        xq = np.zeros((LQG, D), np.float32)
        xk = np.zeros((NKG, D), np.float32)
        xv = np.zeros((NKG, D), np.float32)
        mqa = np.zeros((9, LQG), np.float32)
        mka = np.zeros((9, NKG), np.float32)
        mka[8, :] = SQRT_BIG
        pq, sq = [], []
        for c in range(M):
            tq = np.nonzero(aq[b] == c)[0]
            tk = np.nonzero(ak[b] == c)[0]
            xq[qoffs[c]:qoffs[c] + len(tq)] = queries[b, tq]
            xk[koffs[c]:koffs[c] + len(tk)] = keys[b, tk]
            xv[koffs[c]:koffs[c] + len(tk)] = values[b, tk]
            mqa[c, qoffs[c]:qoffs[c] + len(tq)] = SQRT_BIG
            mqa[8, qoffs[c]:qoffs[c] + len(tq)] = -SQRT_BIG
            mka[c, koffs[c]:koffs[c] + len(tk)] = SQRT_BIG
            pq.append(tq)
            sq.append(np.arange(qoffs[c], qoffs[c] + len(tq)))
        perm_q.append(np.concatenate(pq))
        slot_q.append(np.concatenate(sq))
        XQTs.append(np.ascontiguousarray(xq.T).astype(NPDT))
        XKTs.append(np.ascontiguousarray(xk.T).astype(NPDT))
        XVTs.append(np.ascontiguousarray(xv.T).astype(NPDT))
        MQs.append(mqa.astype(NPDT))
        MKs.append(mka.astype(NPDT))

    def pack_w(w):
        # [1024, 256] -> on-chip [128, 8*256] with w[p, d*256+m] = W[d*128+p, m]
        return np.ascontiguousarray(
            w.reshape(8, 128, 256).transpose(1, 0, 2).reshape(128, 2048)).astype(NPDT)

    def pack_wo(w):
        # [256, 1024] -> [128, 2*1024] with wo[p, n*1024+m] = W[n*128+p, m]
        return np.ascontiguousarray(
            w.reshape(2, 128, 1024).transpose(1, 0, 2).reshape(128, 2048)).astype(NPDT)

    in_maps = []
    for core in range(N_CORES):
        b, hg = core // HPC, core % HPC
        cols = slice(hg * HPC * DH, (hg + 1) * HPC * DH)
        in_maps.append({
            "XQT": XQTs[b], "XKT": XKTs[b], "XVT": XVTs[b],
            "WQ": pack_w(Wq[:, cols] * scale),
            "WK": pack_w(Wk[:, cols]),
            "WV": pack_w(Wv[:, cols]),
            "WO": pack_wo(Wo[cols, :]),
            "MQ": MQs[b], "MK": MKs[b],
        })

    res = run_bass_kernel_spmd(nc, in_maps, list(range(N_CORES)), trace=trace)

    out = np.zeros((B, LQ, D), np.float32)
    for b in range(B):
        acc = res.results[b * HPC]["OUT"].astype(np.float32)
        for hg in range(1, HPC):
            acc += res.results[b * HPC + hg]["OUT"].astype(np.float32)
        out[b, perm_q[b]] = acc[slot_q[b]]
    return out, res


def kernel(**inputs):
    out, _ = run(inputs)
    return out


# revision 8
# speedup vs baseline: 1.9705x; 1.9705x over previous
"""MoE clustered attention kernel for Trainium2 (8 NeuronCores).

Problem: B=2, LQ=LK=2048, D=1024, H=16 heads (DH=64), M=8 clusters.
Each query/key token is routed (argmax of X @ Wr) to one of 8 clusters;
attention is only computed within a cluster (block-sparse attention).

Strategy (v3)
-------------
Host side:
  * compute router assignments with numpy fp32,
  * gather tokens by cluster into padded contiguous ranges (queries to
    >=256 and even, keys to multiples of 128) so one SPMD program
    serves both batches,
  * pre-transpose X to [D, L]; pre-pack weights into the on-chip
    [128, 2048] layout so weight DMAs are fully contiguous,
  * append 9 "mask rows" to the per-head qT/kT tensors: the scores
    matmul contracts over 64+9=73 rows and the extra rows add exactly
    0 to same-cluster pairs and exactly -16384 to cross-cluster or
    padded pairs (exp(x-16384) == 0).

Device side (per core; core = batch * 4 + head_group, 4 heads each):
  * per-head qT/kT projections in transposed layout [73, L] with N=512
    moving matmuls; v in natural layout with an appended ones column
    that makes the ctx matmul emit the softmax denominator as row 64,
  * attention iterates cluster-outer / head-inner; per (cluster, head):
    scores into 2-bank PSUM supertiles, exp on ScalarE, ctx matmul,
    then Ln of the denominator row (ScalarE, doubles as the staging
    copy to partition 0) and an unnormalized ctx evacuation (VectorE),
  * per cluster: one batched exp(-x) over the 4 heads' ln-rows gives
    the reciprocals; GpSimd broadcasts each head's row and multiplies
    the ctx block in place (keeping the normalize entirely off the
    Tensor/Scalar/Vector critical path),
  * output projection is interleaved: as soon as a cluster's token
    range is normalized for all 4 heads, the covered 128-token chunks
    are projected (N=512 matmuls) and DMA'd out in bf16.
  * ACT tables are patched so Exp and Ln both resolve to the
    natural_log_exp_and_others set: one table load, no thrashing.
Host sums the 4 head-group partials per batch in fp32 and un-permutes.
"""

import numpy as np
import ml_dtypes

import concourse.bacc as bacc
import concourse.tile as tile
import concourse.mybir as mybir
import concourse.hw_specs as hw_specs
from concourse.bass_utils import run_bass_kernel_spmd

F32 = mybir.dt.float32
BF16 = mybir.dt.bfloat16
EXP = mybir.ActivationFunctionType.Exp
LN = mybir.ActivationFunctionType.Ln
MULT = mybir.AluOpType.mult

H = 16            # total heads
HPC = 4           # heads per core
N_CORES = 8
SQRT_BIG = 128.0  # sqrt(16384); mask contributions are exact powers of two

MMDT = BF16
NPDT = ml_dtypes.bfloat16

# Route Exp and Ln to the one ACT table set that contains both, so the
# table-load insertion pass emits a single load instead of thrashing
# between exp_and_others and natural_log on every call.
_orig_get_activation_tables = hw_specs.get_activation_tables


def _patched_get_activation_tables(arch):
    out = {}
    for name, fns in _orig_get_activation_tables(arch).items():
        fns = set(fns)
        if name != "natural_log_exp_and_others":
            fns.discard(EXP)
            fns.discard(LN)
        out[name] = fns
    return out


bacc.get_activation_tables = _patched_get_activation_tables


def _ceil_to(x, m):
    return (x + m - 1) // m * m


def _plan(aq, ak, M):
    """Common (cross-batch) padded cluster geometry."""
    B = aq.shape[0]
    nq = np.array([[int((aq[b] == c).sum()) for c in range(M)] for b in range(B)])
    nk = np.array([[int((ak[b] == c).sum()) for c in range(M)] for b in range(B)])
    NQP = [max(256, _ceil_to(int(nq[:, c].max()), 2)) for c in range(M)]
    NKP = [_ceil_to(max(128, int(nk[:, c].max())), 128) for c in range(M)]
    qoff = np.concatenate([[0], np.cumsum(NQP)])
    koff = np.concatenate([[0], np.cumsum(NKP)])
    LQG = _ceil_to(int(qoff[-1]), 256)
    NKG = _ceil_to(int(koff[-1]), 256)
    return NQP, NKP, qoff[:-1].tolist(), koff[:-1].tolist(), LQG, NKG


def _build_program(NQP, NKP, qoffs, koffs, LQG, NKG, D):
    nc = bacc.Bacc("TRN2", target_bir_lowering=False, debug=False)
    XQT = nc.dram_tensor("XQT", [D, LQG], MMDT, kind="ExternalInput").ap()
    XKT = nc.dram_tensor("XKT", [D, NKG], MMDT, kind="ExternalInput").ap()
    XVT = nc.dram_tensor("XVT", [D, NKG], MMDT, kind="ExternalInput").ap()
    WQ = nc.dram_tensor("WQ", [128, 2048], MMDT, kind="ExternalInput").ap()
    WK = nc.dram_tensor("WK", [128, 2048], MMDT, kind="ExternalInput").ap()
    WV = nc.dram_tensor("WV", [128, 2048], MMDT, kind="ExternalInput").ap()
    WO = nc.dram_tensor("WO", [128, 2048], MMDT, kind="ExternalInput").ap()
    MQ = nc.dram_tensor("MQ", [9, LQG], MMDT, kind="ExternalInput").ap()
    MK = nc.dram_tensor("MK", [9, NKG], MMDT, kind="ExternalInput").ap()
    OUT = nc.dram_tensor("OUT", [LQG, D], BF16, kind="ExternalOutput").ap()

    ND = D // 128          # contraction chunks (8)
    NVC = NKG // 128       # value token chunks
    M = len(NQP)

    with tile.TileContext(nc) as tc:
        with (
            tc.tile_pool(name="weights", bufs=1) as wpool,
            tc.tile_pool(name="proj_out", bufs=1) as projpool,
            tc.tile_pool(name="psA", bufs=2, space="PSUM") as psA,
            tc.tile_pool(name="psB", bufs=2, space="PSUM") as psB,
            tc.tile_pool(name="psC", bufs=2, space="PSUM") as psC,
        ):
            wq = wpool.tile([128, 2048], MMDT, tag="wq")
            wk = wpool.tile([128, 2048], MMDT, tag="wk")
            wv = wpool.tile([128, 2048], MMDT, tag="wv")
            wo = wpool.tile([128, 2048], MMDT, tag="wo")

            qT = [projpool.tile([73, LQG], MMDT, tag=f"qT{h}", name=f"qT{h}")
                  for h in range(HPC)]
            kT = [projpool.tile([73, NKG], MMDT, tag=f"kT{h}", name=f"kT{h}")
                  for h in range(HPC)]
            vA = projpool.tile([128, NVC * 260], MMDT, tag="vA")

            # first weight first: Q proj's inputs arrive earliest
            nc.sync.dma_start(wq[:], WQ)

            nc.vector.memset(vA[:], 1.0)

            # ---- projections ----
            with tc.tile_pool(name="xin", bufs=3) as xpool:
                def proj_T(xdram, L, wtile, dest):
                    """dest[h][0:64, L] = (W_h.T @ X^T), streamed over L."""
                    for off in range(0, L, 512):
                        w = min(512, L - off)
                        xt = xpool.tile([128, ND, 512], MMDT, tag="xt")
                        nc.sync.dma_start(
                            xt[:, :, :w],
                            xdram.rearrange("(n p) m -> p n m", p=128)[:, :, off:off + w])
                        for pair in range(2):
                            ps = psA.tile([128, 512], F32, tag="psproj")
                            for d in range(ND):
                                nc.tensor.matmul(
                                    ps[:, :w],
                                    wtile[:, d * 256 + pair * 128: d * 256 + (pair + 1) * 128],
                                    xt[:, d, :w],
                                    start=(d == 0), stop=(d == ND - 1))
                            if pair == 0:
                                nc.vector.tensor_copy(dest[0][0:64, off:off + w], ps[0:64, :w])
                                nc.scalar.copy(dest[1][0:64, off:off + w], ps[64:128, :w])
                            else:
                                nc.scalar.copy(dest[2][0:64, off:off + w], ps[0:64, :w])
                                nc.vector.tensor_copy(dest[3][0:64, off:off + w], ps[64:128, :w])

                proj_T(XQT, LQG, wq, qT)
                # mask rows (needed by scores, not by projections)
                for h in range(HPC):
                    nc.sync.dma_start(qT[h][64:73, :], MQ)
                    nc.sync.dma_start(kT[h][64:73, :], MK)
                nc.sync.dma_start(wk[:], WK)
                proj_T(XKT, NKG, wk, kT)

                nc.sync.dma_start(wv[:], WV)
                nc.sync.dma_start(wo[:], WO)
                for off in range(0, NKG, 512):
                    w = min(512, NKG - off)
                    xt = xpool.tile([128, ND, 512], MMDT, tag="xt")
                    nc.sync.dma_start(
                        xt[:, :, :w],
                        XVT.rearrange("(n p) m -> p n m", p=128)[:, :, off:off + w])
                    for sub in range(w // 128):
                        tc128 = off // 128 + sub
                        ps = psA.tile([128, 512], F32, tag="psproj")
                        for d in range(ND):
                            nc.tensor.matmul(ps[:, 0:256],
                                             xt[:, d, sub * 128:(sub + 1) * 128],
                                             wv[:, d * 256:(d + 1) * 256],
                                             start=(d == 0), stop=(d == ND - 1))
                        nc.vector.tensor_copy(
                            vA[:].rearrange("p (c h e) -> p c h e", c=NVC, h=HPC)[:, tc128, :, 0:64],
                            ps[:, 0:256].rearrange("p (h e) -> p h e", h=HPC))

            # ---- clustered attention + interleaved output projection ----
            # ctx lives in small per-cluster tiles so cluster c's output
            # projection never write-after-read blocks cluster c+1's ctx.
            with tc.tile_pool(name="epool", bufs=3) as epool, \
                 tc.tile_pool(name="ccpool", bufs=3) as ccpool, \
                 tc.tile_pool(name="btpool", bufs=2) as btpool, \
                 tc.tile_pool(name="bbpool", bufs=4) as bbpool, \
                 tc.tile_pool(name="outsb", bufs=4) as opool:
                for c in range(M):
                    qo, nqp = qoffs[c], NQP[c]
                    nkc = NKP[c] // 128
                    lnrow = btpool.tile([1, HPC * 512], F32, tag="lnrow")
                    rcp = btpool.tile([1, HPC * 512], F32, tag="rcp")
                    ctxc = [ccpool.tile([128, 512], MMDT, tag=f"cc{p}", name=f"cc{p}")
                            for p in range(2)]
                    for h in range(HPC):
                        pair, rb = h // 2, (h % 2) * 64
                        es, eslice = [], []
                        for ki in range(0, nkc, 2):
                            nk2 = min(2, nkc - ki)
                            ps_s = psB.tile([128, 1024], F32, tag="ps_s")
                            e = epool.tile([128, 1024], MMDT, tag="e")
                            for kj in range(nk2):
                                ko = koffs[c] + (ki + kj) * 128
                                nc.tensor.matmul(
                                    ps_s[:, kj * 512: kj * 512 + nqp],
                                    kT[h][0:73, ko:ko + 128],
                                    qT[h][0:73, qo:qo + nqp],
                                    start=True, stop=True)
                                es.append(e)
                                eslice.append(slice(kj * 512, kj * 512 + nqp))
                            pv = ps_s[:].rearrange("p (b n) -> p b n", b=2)[:, 0:nk2, 0:nqp]
                            ev = e[:].rearrange("p (b n) -> p b n", b=2)[:, 0:nk2, 0:nqp]
                            nc.scalar.activation(ev, pv, EXP)
                        ps_c = psC.tile([128, 512], F32, tag="ps_c")
                        for ki in range(nkc):
                            kc128 = koffs[c] // 128 + ki
                            nc.tensor.matmul(ps_c[:65, :nqp],
                                             vA[:, kc128 * 260 + h * 65: kc128 * 260 + (h + 1) * 65],
                                             es[ki][:, eslice[ki]],
                                             start=(ki == 0), stop=(ki == nkc - 1))
                        # ln(denominator) doubles as the staging copy to
                        # partition 0; ctx rows leave PSUM unnormalized
                        nc.scalar.activation(lnrow[:, h * 512: h * 512 + nqp],
                                             ps_c[64:65, :nqp], LN)
                        nc.vector.tensor_copy(ctxc[pair][rb:rb + 64, :nqp],
                                              ps_c[0:64, :nqp])
                    # one batched exp(-x) -> reciprocals for all 4 heads
                    nc.scalar.activation(
                        rcp[:].rearrange("p (h n) -> p h n", h=HPC)[:, :, :nqp],
                        lnrow[:].rearrange("p (h n) -> p h n", h=HPC)[:, :, :nqp],
                        EXP, scale=-1.0)
                    for h in range(HPC):
                        pair, rb = h // 2, (h % 2) * 64
                        bt = bbpool.tile([128, 512], F32, tag="bt")
                        nc.gpsimd.partition_broadcast(
                            bt[:, :nqp], rcp[:, h * 512: h * 512 + nqp])
                        nc.vector.tensor_tensor(
                            ctxc[pair][rb:rb + 64, :nqp],
                            ctxc[pair][rb:rb + 64, :nqp],
                            bt[rb:rb + 64, :nqp], MULT)
                    # output projection of this cluster's token range
                    for j in range(0, nqp, 128):
                        w = min(128, nqp - j)
                        ob = opool.tile([128, 1024], BF16, tag="ob")
                        for n2 in range(2):
                            ps_o = psA.tile([128, 512], F32, tag="psproj")
                            for pair in range(2):
                                nc.tensor.matmul(
                                    ps_o[:w, :],
                                    ctxc[pair][:, j:j + w],
                                    wo[:, pair * 1024 + n2 * 512:
                                       pair * 1024 + (n2 + 1) * 512],
                                    start=(pair == 0), stop=(pair == 1))
                            if n2:
                                nc.scalar.copy(ob[:w, n2 * 512:(n2 + 1) * 512],
                                               ps_o[:w, :])
                            else:
                                nc.vector.tensor_copy(ob[:w, n2 * 512:(n2 + 1) * 512],
                                                      ps_o[:w, :])
                        nc.sync.dma_start(OUT[qo + j: qo + j + w, :], ob[:w, :])

    nc.compile()
    return nc


_CACHE = {}


def run(inputs, trace=False):
    queries = np.asarray(inputs["queries"], np.float32)
    keys = np.asarray(inputs["keys"], np.float32)
    values = np.asarray(inputs["values"], np.float32)
    Wq = np.asarray(inputs["Wq"], np.float32)
    Wk = np.asarray(inputs["Wk"], np.float32)
    Wv = np.asarray(inputs["Wv"], np.float32)
    Wo = np.asarray(inputs["Wo"], np.float32)
    Wr = np.asarray(inputs["Wr"], np.float32)

    B, LQ, D = queries.shape
    M = Wr.shape[1]
    DH = D // H
    scale = np.float32(1.0 / np.sqrt(DH))

    aq = np.argmax(queries @ Wr, axis=-1)   # [B, LQ]
    ak = np.argmax(keys @ Wr, axis=-1)      # [B, LK]

    NQP, NKP, qoffs, koffs, LQG, NKG = _plan(aq, ak, M)

    key = (tuple(NQP), tuple(NKP), LQG, NKG, D, str(MMDT))
    if key not in _CACHE:
        _CACHE[key] = _build_program(NQP, NKP, qoffs, koffs, LQG, NKG, D)
    nc = _CACHE[key]

    # ---- gather + pad, build per-batch inputs ----
    perm_q = []   # original token ids, per batch, in gathered order
    slot_q = []   # gathered positions of those tokens
    XQTs, XKTs, XVTs, MQs, MKs = [], [], [], [], []
    for b in range(B):
        xq = np.zeros((LQG, D), np.float32)
        xk = np.zeros((NKG, D), np.float32)
        xv = np.zeros((NKG, D), np.float32)
        mqa = np.zeros((9, LQG), np.float32)
        mka = np.zeros((9, NKG), np.float32)
        mka[8, :] = SQRT_BIG
        pq, sq = [], []
        for c in range(M):
            tq = np.nonzero(aq[b] == c)[0]
            tk = np.nonzero(ak[b] == c)[0]
            xq[qoffs[c]:qoffs[c] + len(tq)] = queries[b, tq]
            xk[koffs[c]:koffs[c] + len(tk)] = keys[b, tk]
            xv[koffs[c]:koffs[c] + len(tk)] = values[b, tk]
            mqa[c, qoffs[c]:qoffs[c] + len(tq)] = SQRT_BIG
            mqa[8, qoffs[c]:qoffs[c] + len(tq)] = -SQRT_BIG
            mka[c, koffs[c]:koffs[c] + len(tk)] = SQRT_BIG
            pq.append(tq)
            sq.append(np.arange(qoffs[c], qoffs[c] + len(tq)))
        perm_q.append(np.concatenate(pq))
        slot_q.append(np.concatenate(sq))
        XQTs.append(np.ascontiguousarray(xq.T).astype(NPDT))
        XKTs.append(np.ascontiguousarray(xk.T).astype(NPDT))
        XVTs.append(np.ascontiguousarray(xv.T).astype(NPDT))
        MQs.append(mqa.astype(NPDT))
        MKs.append(mka.astype(NPDT))

    def pack_w(w):
        # [1024, 256] -> on-chip [128, 8*256] with w[p, d*256+m] = W[d*128+p, m]
        return np.ascontiguousarray(
            w.reshape(8, 128, 256).transpose(1, 0, 2).reshape(128, 2048)).astype(NPDT)

    def pack_wo(w):
        # [256, 1024] -> [128, 2*1024] with wo[p, n*1024+m] = W[n*128+p, m]
        return np.ascontiguousarray(
            w.reshape(2, 128, 1024).transpose(1, 0, 2).reshape(128, 2048)).astype(NPDT)

    in_maps = []
    for core in range(N_CORES):
        b, hg = core // HPC, core % HPC
        cols = slice(hg * HPC * DH, (hg + 1) * HPC * DH)
        in_maps.append({
            "XQT": XQTs[b], "XKT": XKTs[b], "XVT": XVTs[b],
            "WQ": pack_w(Wq[:, cols] * scale),
            "WK": pack_w(Wk[:, cols]),
            "WV": pack_w(Wv[:, cols]),
            "WO": pack_wo(Wo[cols, :]),
            "MQ": MQs[b], "MK": MKs[b],
        })

    res = run_bass_kernel_spmd(nc, in_maps, list(range(N_CORES)), trace=trace)

    out = np.zeros((B, LQ, D), np.float32)
    for b in range(B):
        acc = res.results[b * HPC]["OUT"].astype(np.float32)
        for hg in range(1, HPC):
            acc += res.results[b * HPC + hg]["OUT"].astype(np.float32)
        out[b, perm_q[b]] = acc[slot_q[b]]
    return out, res


def kernel(**inputs):
    out, _ = run(inputs)
    return out
